# revision 38
# baseline (speedup 1.0000x reference)
"""Trainium2 Bass kernel: MultiHeadSelfAttention (B=1, S=4096, D=512, H=8, DK=DV=64)
with fc_out applied twice.

Sharding: sequence-sharded across 8 cores (512 queries per core). Every core
receives the FULL keys/values (pre-transposed, bf16) and redundantly computes
the full K^T / V projections on-device (cheaper than an AllGather, whose entry
barrier + transfer measured ~100us); attention + the two output projections run
on the core's own 512-query chunk. Host concatenates the 8 output chunks.

Layout notes:
  - scores^T tiles [seq_k(128) x seq_q(512)] come out of PE via lhsT=K^T block,
    rhs=q^T. Both are zero-padded from d=64 to K=128 partitions: K=64 matmuls
    never trip the PE HAM activity monitor, pinning the clock to 1.2 GHz
    (measured); K=128 with zero rows sustains 2.4 GHz.
  - softmax denominator via a ones-column appended to each head's V (stride
    65): attn@V gives [65, 512] per head = output^T rows + exp-sum row.
  - output returned TRANSPOSED ([D, CH]); host un-transposes. This removes all
    PE transposes + DVE copies from the tail; fc0/fc1/bias/DMA pipeline per
    128-dim chunk instead.
"""
import sys, functools
sys.path.insert(0, "/opt/trn_rl_repo")
if "/root/.axon_site" not in sys.path:
    sys.path.insert(0, "/root/.axon_site")
import numpy as np
import ml_dtypes

import concourse.bass as bass
import concourse.tile as tile
from concourse import bacc, mybir, masks
from concourse.bass_utils import run_bass_kernel_spmd

NCORES = 8
S, D, H, DK = 4096, 512, 8, 64
CH = S // NCORES            # 512 sequence rows per core
VW = H * (DK + 1)           # 520: v row width incl. ones columns
JT = S // 128               # 32 seq_k tiles
CHUNK = 3                   # j-tiles per exp batch ([128,1536] psum, 3 banks x 2)

F32 = mybir.dt.float32
BF16 = mybir.dt.bfloat16
EXP = mybir.ActivationFunctionType.Exp


def _build_program():
    nc = bacc.Bacc("TRN2", target_bir_lowering=False, debug=False,
                   num_devices=NCORES)

    xqT = nc.dram_tensor("xqT", [D, CH], BF16, kind="ExternalInput")
    keysT = nc.dram_tensor("keysT", [D, S], BF16, kind="ExternalInput")
    valsT = nc.dram_tensor("valsT", [D, S], BF16, kind="ExternalInput")
    Wq = nc.dram_tensor("Wq", [D, D], BF16, kind="ExternalInput")
    Wk = nc.dram_tensor("Wk", [D, D], BF16, kind="ExternalInput")
    Wv = nc.dram_tensor("Wv", [D, D], BF16, kind="ExternalInput")
    # host folds the two fc_out applications: W2 = Wo@Wo, b2 = bo@Wo + bo
    Wo = nc.dram_tensor("Wo", [D, D], BF16, kind="ExternalInput")
    bo = nc.dram_tensor("bo", [D], F32, kind="ExternalInput")
    # output^T: [D, CH]; host transposes back
    yT = nc.dram_tensor("yT", [D, CH], F32, kind="ExternalOutput")

    with tile.TileContext(nc) as tc:
        with tc.tile_pool(name="persist", bufs=1) as pp, \
             tc.tile_pool(name="kv", bufs=1) as kvp:

            Wo_sb = pp.tile([128, 2048], BF16, tag="wo")
            Wk_sb = pp.tile([128, 2048], BF16, tag="wk")
            Wv_sb = pp.tile([128, 2048], BF16, tag="wv")
            # two bias tiles, one per bias engine (ACT / DVE), so the tail
            # bias-adds share no tiles across engines
            boA = pp.tile([128, 2], F32, tag="boA")
            boB = pp.tile([128, 2], F32, tag="boB")
            ones64 = pp.tile([1, 64], BF16, tag="on")
            warm_sb = pp.tile([128, 256], BF16, tag="warm")
            o2p = [pp.tile([128, 512], F32, tag=f"o2{m}", name=f"o2_{m}")
                   for m in range(4)]
            # q^T per head: even heads rows 0-63 (zeros below), odd heads rows
            # 64-127 (zeros above) - matches the packed K^T pair layout
            qTz_sb = pp.tile([128, H * 512], BF16, tag="qt")
            # attention output^T, one tile per head pair so fc partials can
            # start as soon as a pair completes
            attTp = [pp.tile([128, 512], BF16, tag=f"att{p}", name=f"attT{p}")
                     for p in range(4)]
            # K^T packed head pairs: head 2p on rows 0-63, head 2p+1 on 64-127;
            # the zero padding that keeps scores at K=128 lives in qTz instead
            KTp = [kvp.tile([128, S], BF16, tag=f"kt{p}", name=f"KT{p}")
                   for p in range(H // 2)]
            # V natural [seq, head-stripes of 65 (64 + ones col)]
            V_sb = kvp.tile([128, JT * VW], BF16, tag="v")

            # zero pads + ones columns on gpsimd (keeps DVE free)
            nc.vector.memset(qTz_sb[:], 0.0)
            nc.vector.memset(ones64[:], 1.0)
            nc.vector.memset(warm_sb[:], 0.0)
            nc.gpsimd.memset(
                V_sb[:].rearrange("p (j h x) -> p j h x", j=JT, h=H, x=DK + 1)
                [:, :, :, DK:DK + 1], 1.0)

            with tc.tile_pool(name="kstage", bufs=1) as ksp, \
                 tc.tile_pool(name="pt", bufs=6) as ptp, \
                 tc.tile_pool(name="rc", bufs=2) as rcp, \
                 tc.tile_pool(name="ps_av", bufs=1, space="PSUM") as psav:

                def staged_load(dst_sb, src_dram, eng, nchunks=8):
                    w = S // nchunks
                    dst = dst_sb[:].rearrange("p (k s) -> p k s", k=4)
                    srcv = src_dram.ap().rearrange("(k p) s -> p k s", p=128)
                    for ci in range(nchunks):
                        eng.dma_start(dst[:, :, w * ci:w * ci + w],
                                      srcv[:, :, w * ci:w * ci + w])

                def q_proj(pool):
                    for m in range(4):
                        ps = pool.tile([128, 512], F32, tag="bg", name=f"qp{m}")
                        for k in range(4):
                            nc.tensor.matmul(
                                ps[:], lhsT=Wq_sb[:, 512 * k + 128 * m:512 * k + 128 * m + 128],
                                rhs=xqT_sb[:, 512 * k:512 * k + 512],
                                start=(k == 0), stop=(k == 3))
                        nc.vector.tensor_copy(
                            qTz_sb[0:64, 512 * (2 * m):512 * (2 * m) + 512], ps[0:64, :])
                        nc.vector.tensor_copy(
                            qTz_sb[64:128, 512 * (2 * m + 1):512 * (2 * m + 1) + 512],
                            ps[64:128, :])

                def v_proj_group(j, pool):
                    ps = pool.tile([128, 512], F32, tag="bg", name=f"vp{j}")
                    for k in range(4):
                        nc.tensor.matmul(
                            ps[:], lhsT=vst[:, S * k + 128 * j:S * k + 128 * j + 128],
                            rhs=Wv_sb[:, 512 * k:512 * k + 512],
                            start=(k == 0), stop=(k == 3))
                    dst = V_sb[:, VW * j:VW * j + VW].rearrange(
                        "p (h x) -> p h x", h=H, x=DK + 1)[:, :, 0:DK]
                    nc.vector.tensor_copy(
                        dst, ps[:].rearrange("p (h x) -> p h x", h=H, x=DK))

                def k_proj_group(m, sc, pool, tag, copy_eng=None):
                    ps = pool.tile([128, 512], F32, tag=tag, name=f"kp{m}_{sc}")
                    for k in range(4):
                        nc.tensor.matmul(
                            ps[:], lhsT=Wk_sb[:, 512 * k + 128 * m:512 * k + 128 * m + 128],
                            rhs=kst0[:, S * k + 512 * sc:S * k + 512 * sc + 512],
                            start=(k == 0), stop=(k == 3))
                    dst = KTp[m][:, 512 * sc:512 * sc + 512]
                    if copy_eng == "act":
                        # ACT is idle between heads; copying there keeps the
                        # single kproj psum bank draining without queueing
                        # behind the DVE normalization chain
                        nc.scalar.copy(dst, ps[:])
                    else:
                        nc.vector.tensor_copy(dst, ps[:])

                def attention_head(h, pool_sc, chunk, drip=None):
                    q_ap = qTz_sb[:, 512 * h:512 * h + 512]
                    av = psav.tile([65, 512], F32, tag="av", name=f"av{h}")

                    def attn_v(js, pt):
                        for i, j in enumerate(js):
                            nc.tensor.matmul(
                                av[:],
                                lhsT=V_sb[:, VW * j + 65 * h:VW * j + 65 * h + 65],
                                rhs=pt[:, 512 * i:512 * i + 512],
                                start=(j == 0), stop=(j == JT - 1))

                    pend = None  # attn@V lags one chunk so scores stay ahead of ACT
                    for c in range((JT + chunk - 1) // chunk):
                        js = list(range(chunk * c, min(chunk * c + chunk, JT)))
                        ps = pool_sc.tile([128, 512 * chunk], F32, tag="sc",
                                          name=f"sc{h}_{c}")
                        pt = ptp.tile([128, 512 * chunk], BF16, tag="pt",
                                      name=f"pt{h}_{c}")
                        for i, j in enumerate(js):
                            nc.tensor.matmul(
                                ps[:, 512 * i:512 * i + 512],
                                lhsT=KTp[h // 2][:, 128 * j:128 * j + 128],
                                rhs=q_ap, start=True, stop=True)
                        w = 512 * len(js)
                        nc.scalar.activation(pt[:, 0:w], ps[:, 0:w], EXP, scale=0.125)
                        if drip is not None:
                            drip(c)
                        if pend is not None:
                            attn_v(*pend)
                        pend = (js, pt)
                    last_pt = pend[1]
                    attn_v(*pend)
                    hp, hl = h // 2, h % 2
                    att_dst = attTp[hp][64 * hl:64 * hl + 64, :]
                    if h == 7:
                        # critical-path tail: read av psum directly, broadcast
                        # the denominator via a K=1 matmul on the (idle) PE
                        rtmpb = rcp.tile([1, 512], BF16, tag="rt", name="rt7")
                        nc.vector.tensor_copy(rtmpb[:], av[64:65, :])
                        rbb = pskp.tile([64, 512], F32, tag="kp", name="rbb7")
                        nc.tensor.matmul(rbb[:], lhsT=ones64[:], rhs=rtmpb[:],
                                         start=True, stop=True)
                        rb2 = rcp.tile([64, 512], F32, tag="rb2", name="rb27")
                        nc.vector.reciprocal_approx_fast(out=rb2[:], in_=rbb[:])
                        nc.vector.tensor_mul(att_dst, av[0:64, :], rb2[:])
                        return last_pt
                    # copy psum accumulator out immediately so the bank frees
                    avc = rcp.tile([65, 512], F32, tag="avc", name=f"avc{h}")
                    rbc = rcp.tile([64, 512], F32, tag="rb", name=f"rb{h}")
                    rtmp = rcp.tile([1, 512], F32, tag="rt", name=f"rt{h}")
                    nc.vector.tensor_copy(avc[:], av[:])
                    nc.vector.tensor_copy(rtmp[:], av[64:65, :])
                    rb2 = rcp.tile([64, 512], F32, tag="rb2", name=f"rb2{h}")
                    nc.gpsimd.partition_broadcast(rbc[:], rtmp[:])
                    nc.vector.reciprocal_approx_fast(out=rb2[:], in_=rbc[:])
                    nc.vector.tensor_mul(att_dst, avc[0:64, :], rb2[:])

                # ---- scope A: q proj, K0, head 0 with V-proj dripped in ----
                pfx = tc.tile_pool(name="xin", bufs=1)
                xp = pfx.__enter__()
                scA = tc.tile_pool(name="ps_scA", bufs=2, space="PSUM")
                pssc2 = scA.__enter__()
                bgp_cm = tc.tile_pool(name="ps_bg", bufs=2, space="PSUM")
                bgp = bgp_cm.__enter__()

                Wq_sb = xp.tile([128, 2048], BF16, tag="wq")
                xqT_sb = xp.tile([128, 2048], BF16, tag="xq")
                vst = xp.tile([128, 4 * S], BF16, tag="vs", name="vstage")
                kst0 = ksp.tile([128, 4 * S], BF16, tag="ks", name="kstage0")
                # split first loads so q_proj can begin on the first halves;
                # K/V staging issues from otherwise-idle engine queues so the
                # sync engine's serial DMA-issue cost doesn't pace arrivals
                xq_d = xqT_sb[:].rearrange("p (k s) -> p k s", k=4)
                xq_s = xqT.ap().rearrange("(k p) s -> p k s", p=128)
                wq_d = Wq_sb[:].rearrange("p (k n) -> p k n", k=4)
                wq_s = Wq.ap().rearrange("(k p) n -> p k n", p=128)
                nc.sync.dma_start(xq_d[:, 0:2, :], xq_s[:, 0:2, :])
                nc.sync.dma_start(wq_d[:, 0:2, :], wq_s[:, 0:2, :])
                nc.sync.dma_start(xq_d[:, 2:4, :], xq_s[:, 2:4, :])
                nc.sync.dma_start(wq_d[:, 2:4, :], wq_s[:, 2:4, :])
                nc.sync.dma_start(
                    Wk_sb[:].rearrange("p (k n) -> p k n", k=4),
                    Wk.ap().rearrange("(k p) n -> p k n", p=128))
                nc.sync.dma_start(
                    Wv_sb[:].rearrange("p (k n) -> p k n", k=4),
                    Wv.ap().rearrange("(k p) n -> p k n", p=128))
                # K/V chunks interleaved to match head-0's consumption order:
                # chunk c of head 0 consumes kst c+1 and vst c//2
                kd = kst0[:].rearrange("p (k s) -> p k s", k=4)
                ks_ = keysT.ap().rearrange("(k p) s -> p k s", p=128)
                vd = vst[:].rearrange("p (k s) -> p k s", k=4)
                vs_ = valsT.ap().rearrange("(k p) s -> p k s", p=128)
                order = [(0, 0), (0, 1), (1, 0), (0, 2), (0, 3), (1, 1),
                         (0, 4), (0, 5), (1, 2), (0, 6), (0, 7), (1, 3),
                         (1, 4), (1, 5), (1, 6), (1, 7)]
                for which, ci in order:
                    dst, src = (kd, ks_) if which == 0 else (vd, vs_)
                    nc.sync.dma_start(dst[:, :, 512 * ci:512 * ci + 512],
                                      src[:, :, 512 * ci:512 * ci + 512])
                nc.sync.dma_start(
                    Wo_sb[:].rearrange("p (k n) -> p k n", k=4),
                    Wo.ap().rearrange("(k p) n -> p k n", p=128))
                bo_src = bo.ap().rearrange("(m p) -> p m", p=128)
                nc.sync.dma_start(boA[:], bo_src[:, 0:2])
                nc.sync.dma_start(boB[:], bo_src[:, 2:4])

                # PE p-state warmup: matmuls run at ~1.2 GHz until the PE has
                # been continuously busy ~3us, and the first real matmul can't
                # start until its DMA lands (~12us). These throwaway 256-col
                # matmuls are emitted BEFORE q_proj so they unconditionally
                # lead the PE stream, ramping the clock while DMA streams in.
                wrm_cm = tc.tile_pool(name="ps_warm", bufs=1, space="PSUM")
                wrm = wrm_cm.__enter__()
                warm_ps = wrm.tile([64, 256], F32, tag="w", name="warm_ps")
                for _ in range(38):
                    nc.tensor.matmul(
                        warm_ps[:], lhsT=warm_sb[:, 0:64],
                        rhs=warm_sb[:, 0:256], start=True, stop=True)
                wrm_cm.__exit__(None, None, None)

                q_proj(bgp)
                k_proj_group(0, 0, bgp, "bg")

                def drip_kv(c):
                    # K-pair-0 groups dripped just ahead of the chunks that
                    # need them (chunk c+1 needs sc <= (2c+3)//4 <= c+1), so
                    # the PE isn't parked on not-yet-loaded kst chunks
                    if c + 1 < 8:
                        k_proj_group(0, c + 1, bgp, "bg")
                    for j in (2 * c, 2 * c + 1):
                        if j < JT:
                            v_proj_group(j, bgp)

                attention_head(0, pssc2, 2, drip_kv)

                bgp_cm.__exit__(None, None, None)
                scA.__exit__(None, None, None)
                pfx.__exit__(None, None, None)

                # ---- scope B: heads 1-7; K pair m batched before head 2m;
                # fused fc (W2 = Wo@Wo folded on host) emitted in-scope so its
                # psum accumulators carve the scores-pool banks with no
                # scope-exit barrier: k=0..2 partials fill the PE gap while
                # head 7's normalization chain finishes attTp[3] ----
                with tc.tile_pool(name="ps_sc", bufs=2, space="PSUM") as pssc, \
                     tc.tile_pool(name="ps_kp", bufs=1, space="PSUM") as pskp:
                    # heads are ACT-paced (exp chain ~16.1us vs PE ~13.65us),
                    # so K pair m's 8 proj groups drip into the PE slack of
                    # heads 2m-1 (late chunks) and 2m (alternating chunks)
                    # instead of running as pure-PE batches between heads
                    sched = {h: {} for h in range(1, 8)}
                    for m in (1, 2, 3):
                        for i in range(4):
                            sched[2 * m - 1][7 + i] = (m, i)
                            sched[2 * m][2 + 2 * i] = (m, 4 + i)

                    def make_drip_k(h):
                        hs = sched.get(h, {})
                        if not hs:
                            return None
                        def drip(c):
                            if c in hs:
                                m, sc = hs[c]
                                k_proj_group(m, sc, pskp, "kp")
                        return drip

                    for h in range(1, 7):
                        attention_head(h, pssc, CHUNK, make_drip_k(h))
                    gate_pt = attention_head(7, pssc, CHUNK)

                    fcA = pssc.tile([128, 1536], F32, tag="sc", name="fcA")
                    fcB = pssc.tile([128, 1536], F32, tag="sc", name="fcB")
                    # gate the fc partials on head 7's last exp chunk: without
                    # this they become ready early and stuff the PE hardware
                    # queue ahead of head 7's final attn@V matmuls, delaying
                    # the whole normalization chain
                    nc.vector.tensor_copy(fcA[0:1, 0:1], gate_pt[0:1, 0:1])
                    nc.vector.tensor_copy(fcB[0:1, 0:1], gate_pt[0:1, 0:1])
                    fct = [fcA[:, 0:512], fcA[:, 512:1024],
                           fcB[:, 0:512], fcB[:, 512:1024]]
                    yT_d = yT.ap().rearrange("(m p) f -> p m f", m=4, p=128)
                    for m in range(4):
                        for k in range(3):
                            nc.tensor.matmul(
                                fct[m], lhsT=Wo_sb[:, 512 * k + 128 * m:512 * k + 128 * m + 128],
                                rhs=attTp[k][:], start=(k == 0), stop=False,
                                skip_group_check=True)
                    for m in range(4):
                        nc.tensor.matmul(
                            fct[m], lhsT=Wo_sb[:, 512 * 3 + 128 * m:512 * 3 + 128 * m + 128],
                            rhs=attTp[3][:], start=False, stop=True,
                            skip_group_check=True)
                    # biases paired same-engine per psum tile (fcA -> ACT,
                    # fcB -> DVE) with per-engine bias tiles: the tail stages
                    # share no tiles across engines, so nothing serializes
                    nc.scalar.add(o2p[0][:], fct[0], boA[:, 0:1])
                    nc.scalar.add(o2p[1][:], fct[1], boA[:, 1:2])
                    nc.vector.tensor_scalar_add(o2p[2][:], fct[2], boB[:, 0:1])
                    nc.vector.tensor_scalar_add(o2p[3][:], fct[3], boB[:, 1:2])
                    for m, eng in ((0, nc.sync), (1, nc.sync),
                                   (2, nc.gpsimd), (3, nc.gpsimd)):
                        eng.dma_start(yT_d[:, m, :], o2p[m][:])

    nc.compile()
    return nc


@functools.lru_cache(maxsize=1)
def _get_program():
    return _build_program()


def _make_in_maps(queries, keys, values, Wq, Wk, Wv, Wo, bo):
    q = np.asarray(queries, np.float32).reshape(S, D)
    kT = np.ascontiguousarray(np.asarray(keys, np.float32).reshape(S, D).T
                              ).astype(ml_dtypes.bfloat16)
    vT = np.ascontiguousarray(np.asarray(values, np.float32).reshape(S, D).T
                              ).astype(ml_dtypes.bfloat16)
    Wq = np.ascontiguousarray(np.asarray(Wq, np.float32)).astype(ml_dtypes.bfloat16)
    Wk = np.ascontiguousarray(np.asarray(Wk, np.float32)).astype(ml_dtypes.bfloat16)
    Wv = np.ascontiguousarray(np.asarray(Wv, np.float32)).astype(ml_dtypes.bfloat16)
    # fold the two fc_out applications into one: y = a@(Wo@Wo) + (bo@Wo + bo)
    Wo64 = np.asarray(Wo, np.float64)
    bo64 = np.asarray(bo, np.float64)
    Wo = np.ascontiguousarray(Wo64 @ Wo64).astype(ml_dtypes.bfloat16)
    bo = np.ascontiguousarray(bo64 @ Wo64 + bo64).astype(np.float32)
    in_maps = []
    for c in range(NCORES):
        in_maps.append({
            "xqT": np.ascontiguousarray(q[c * CH:(c + 1) * CH].T).astype(ml_dtypes.bfloat16),
            "keysT": kT, "valsT": vT,
            "Wq": Wq, "Wk": Wk, "Wv": Wv, "Wo": Wo, "bo": bo,
        })
    return in_maps


def _run(in_maps, **kw):
    nc = _get_program()
    return run_bass_kernel_spmd(nc, in_maps, core_ids=list(range(NCORES)), **kw)


def kernel(queries, keys, values, Wq, Wk, Wv, Wo, bo):
    res = _run(_make_in_maps(queries, keys, values, Wq, Wk, Wv, Wo, bo))
    out = np.concatenate([res.results[c]["yT"].T for c in range(NCORES)], axis=0)
    return out.reshape(1, S, D)


def run_traced(queries, keys, values, Wq, Wk, Wv, Wo, bo):
    """Like kernel() but with NTFF profiling; returns (output, BassKernelResults)."""
    import types
    import trn_agent_boot.trn_boot as _tb
    from concourse import bass_utils
    hook = _tb._ntff_profile_via_ctypes("/opt/axon/libaxon_pjrt.so")
    mod = types.ModuleType("antenv.axon_hooks")
    mod.get_axon_ntff_profile_hook = lambda: hook
    sys.modules["antenv.axon_hooks"] = mod
    bass_utils.upload_artifacts = lambda tmpdir: tmpdir
    res = _run(_make_in_maps(queries, keys, values, Wq, Wk, Wv, Wo, bo), trace=True)
    out = np.concatenate([res.results[c]["yT"].T for c in range(NCORES)], axis=0)
    return out.reshape(1, S, D), res


# revision 40
# speedup vs baseline: 1.2181x; 1.2181x over previous
"""Trainium2 Bass kernel: MultiHeadSelfAttention (B=1, S=4096, D=512, H=8, DK=DV=64)
with fc_out applied twice.

Sharding: sequence-sharded across 8 cores (512 queries per core). Every core
receives the FULL keys/values (pre-transposed, bf16) and redundantly computes
the full K^T / V projections on-device (cheaper than an AllGather, whose entry
barrier + transfer measured ~100us); attention + the two output projections run
on the core's own 512-query chunk. Host concatenates the 8 output chunks.

Layout notes:
  - scores^T tiles [seq_k(128) x seq_q(512)] come out of PE via lhsT=K^T block,
    rhs=q^T. Both are zero-padded from d=64 to K=128 partitions: K=64 matmuls
    never trip the PE HAM activity monitor, pinning the clock to 1.2 GHz
    (measured); K=128 with zero rows sustains 2.4 GHz.
  - softmax denominator via a ones-column appended to each head's V (stride
    65): attn@V gives [65, 512] per head = output^T rows + exp-sum row.
  - output returned TRANSPOSED ([D, CH]); host un-transposes. This removes all
    PE transposes + DVE copies from the tail; fc0/fc1/bias/DMA pipeline per
    128-dim chunk instead.
"""
import sys, functools
sys.path.insert(0, "/opt/trn_rl_repo")
if "/root/.axon_site" not in sys.path:
    sys.path.insert(0, "/root/.axon_site")
import numpy as np
import ml_dtypes

import concourse.bass as bass
import concourse.tile as tile
from concourse import bacc, mybir, masks
from concourse.bass_utils import run_bass_kernel_spmd

NCORES = 8
S, D, H, DK = 4096, 512, 8, 64
CH = S // NCORES            # 512 sequence rows per core
VW = H * (DK + 1)           # 520: v row width incl. ones columns
JT = S // 128               # 32 seq_k tiles
CHUNK = 3                   # j-tiles per exp batch ([128,1536] psum, 3 banks x 2)

F32 = mybir.dt.float32
BF16 = mybir.dt.bfloat16
EXP = mybir.ActivationFunctionType.Exp


def _build_program():
    nc = bacc.Bacc("TRN2", target_bir_lowering=False, debug=False,
                   num_devices=NCORES)

    xqT = nc.dram_tensor("xqT", [D, CH], BF16, kind="ExternalInput")
    keysT = nc.dram_tensor("keysT", [D, S], BF16, kind="ExternalInput")
    valsT = nc.dram_tensor("valsT", [D, S], BF16, kind="ExternalInput")
    Wq = nc.dram_tensor("Wq", [D, D], BF16, kind="ExternalInput")
    Wk = nc.dram_tensor("Wk", [D, D], BF16, kind="ExternalInput")
    Wv = nc.dram_tensor("Wv", [D, D], BF16, kind="ExternalInput")
    # host folds the two fc_out applications: W2 = Wo@Wo, b2 = bo@Wo + bo
    Wo = nc.dram_tensor("Wo", [D, D], BF16, kind="ExternalInput")
    bo = nc.dram_tensor("bo", [D], F32, kind="ExternalInput")
    # output^T: [D, CH]; host transposes back
    yT = nc.dram_tensor("yT", [D, CH], F32, kind="ExternalOutput")

    with tile.TileContext(nc) as tc:
        with tc.tile_pool(name="persist", bufs=1) as pp, \
             tc.tile_pool(name="kv", bufs=1) as kvp:

            Wo_sb = pp.tile([128, 2048], BF16, tag="wo")
            Wk_sb = pp.tile([128, 2048], BF16, tag="wk")
            Wv_sb = pp.tile([128, 2048], BF16, tag="wv")
            # two bias tiles, one per bias engine (ACT / DVE), so the tail
            # bias-adds share no tiles across engines
            boA = pp.tile([128, 2], F32, tag="boA")
            boB = pp.tile([128, 2], F32, tag="boB")
            ones64 = pp.tile([1, 64], BF16, tag="on")
            warm_sb = pp.tile([128, 256], BF16, tag="warm")
            o2p = [pp.tile([128, 512], F32, tag=f"o2{m}", name=f"o2_{m}")
                   for m in range(4)]
            # q^T per head: even heads rows 0-63 (zeros below), odd heads rows
            # 64-127 (zeros above) - matches the packed K^T pair layout
            qTz_sb = pp.tile([128, H * 512], BF16, tag="qt")
            # attention output^T, one tile per head pair so fc partials can
            # start as soon as a pair completes
            attTp = [pp.tile([128, 512], BF16, tag=f"att{p}", name=f"attT{p}")
                     for p in range(4)]
            # K^T packed head pairs: head 2p on rows 0-63, head 2p+1 on 64-127;
            # the zero padding that keeps scores at K=128 lives in qTz instead
            KTp = [kvp.tile([128, S], BF16, tag=f"kt{p}", name=f"KT{p}")
                   for p in range(H // 2)]
            # V natural [seq, head-stripes of 65 (64 + ones col)]
            V_sb = kvp.tile([128, JT * VW], BF16, tag="v")

            # zero pads + ones columns on gpsimd (keeps DVE free)
            nc.vector.memset(warm_sb[:], 0.0)
            nc.vector.memset(qTz_sb[:], 0.0)
            nc.vector.memset(ones64[:], 1.0)
            nc.gpsimd.memset(
                V_sb[:].rearrange("p (j h x) -> p j h x", j=JT, h=H, x=DK + 1)
                [:, :, :, DK:DK + 1], 1.0)

            with tc.tile_pool(name="kstage", bufs=1) as ksp, \
                 tc.tile_pool(name="pt", bufs=6) as ptp, \
                 tc.tile_pool(name="rc", bufs=2) as rcp, \
                 tc.tile_pool(name="ps_av", bufs=1, space="PSUM") as psav:

                def staged_load(dst_sb, src_dram, eng, nchunks=8):
                    w = S // nchunks
                    dst = dst_sb[:].rearrange("p (k s) -> p k s", k=4)
                    srcv = src_dram.ap().rearrange("(k p) s -> p k s", p=128)
                    for ci in range(nchunks):
                        eng.dma_start(dst[:, :, w * ci:w * ci + w],
                                      srcv[:, :, w * ci:w * ci + w])

                def q_proj(pool):
                    for m in range(4):
                        ps = pool.tile([128, 512], F32, tag="bg", name=f"qp{m}")
                        for k in range(4):
                            nc.tensor.matmul(
                                ps[:], lhsT=Wq_sb[:, 512 * k + 128 * m:512 * k + 128 * m + 128],
                                rhs=xqT_sb[:, 512 * k:512 * k + 512],
                                start=(k == 0), stop=(k == 3))
                        nc.vector.tensor_copy(
                            qTz_sb[0:64, 512 * (2 * m):512 * (2 * m) + 512], ps[0:64, :])
                        nc.vector.tensor_copy(
                            qTz_sb[64:128, 512 * (2 * m + 1):512 * (2 * m + 1) + 512],
                            ps[64:128, :])

                def v_proj_group(j, pool):
                    ps = pool.tile([128, 512], F32, tag="bg", name=f"vp{j}")
                    for k in range(4):
                        nc.tensor.matmul(
                            ps[:], lhsT=vst[:, S * k + 128 * j:S * k + 128 * j + 128],
                            rhs=Wv_sb[:, 512 * k:512 * k + 512],
                            start=(k == 0), stop=(k == 3))
                    dst = V_sb[:, VW * j:VW * j + VW].rearrange(
                        "p (h x) -> p h x", h=H, x=DK + 1)[:, :, 0:DK]
                    nc.vector.tensor_copy(
                        dst, ps[:].rearrange("p (h x) -> p h x", h=H, x=DK))

                def k_proj_group(m, sc, pool, tag, copy_eng=None):
                    ps = pool.tile([128, 512], F32, tag=tag, name=f"kp{m}_{sc}")
                    for k in range(4):
                        nc.tensor.matmul(
                            ps[:], lhsT=Wk_sb[:, 512 * k + 128 * m:512 * k + 128 * m + 128],
                            rhs=kst0[:, S * k + 512 * sc:S * k + 512 * sc + 512],
                            start=(k == 0), stop=(k == 3))
                    dst = KTp[m][:, 512 * sc:512 * sc + 512]
                    if copy_eng == "act":
                        # ACT is idle between heads; copying there keeps the
                        # single kproj psum bank draining without queueing
                        # behind the DVE normalization chain
                        nc.scalar.copy(dst, ps[:])
                    else:
                        nc.vector.tensor_copy(dst, ps[:])

                def attention_head(h, pool_sc, chunk, drip=None):
                    q_ap = qTz_sb[:, 512 * h:512 * h + 512]
                    av = psav.tile([65, 512], F32, tag="av", name=f"av{h}")

                    def attn_v(js, pt):
                        for i, j in enumerate(js):
                            nc.tensor.matmul(
                                av[:],
                                lhsT=V_sb[:, VW * j + 65 * h:VW * j + 65 * h + 65],
                                rhs=pt[:, 512 * i:512 * i + 512],
                                start=(j == 0), stop=(j == JT - 1))

                    pend = None  # attn@V lags one chunk so scores stay ahead of ACT
                    for c in range((JT + chunk - 1) // chunk):
                        js = list(range(chunk * c, min(chunk * c + chunk, JT)))
                        ps = pool_sc.tile([128, 512 * chunk], F32, tag="sc",
                                          name=f"sc{h}_{c}")
                        pt = ptp.tile([128, 512 * chunk], BF16, tag="pt",
                                      name=f"pt{h}_{c}")
                        for i, j in enumerate(js):
                            nc.tensor.matmul(
                                ps[:, 512 * i:512 * i + 512],
                                lhsT=KTp[h // 2][:, 128 * j:128 * j + 128],
                                rhs=q_ap, start=True, stop=True)
                        w = 512 * len(js)
                        nc.scalar.activation(pt[:, 0:w], ps[:, 0:w], EXP, scale=0.125)
                        if drip is not None:
                            drip(c)
                        if pend is not None:
                            attn_v(*pend)
                        pend = (js, pt)
                    last_pt = pend[1]
                    attn_v(*pend)
                    hp, hl = h // 2, h % 2
                    att_dst = attTp[hp][64 * hl:64 * hl + 64, :]
                    if h == 7:
                        # critical-path tail: read av psum directly, broadcast
                        # the denominator via a K=1 matmul on the (idle) PE
                        rtmpb = rcp.tile([1, 512], BF16, tag="rt", name="rt7")
                        nc.vector.tensor_copy(rtmpb[:], av[64:65, :])
                        rbb = pskp.tile([64, 512], F32, tag="kp", name="rbb7")
                        nc.tensor.matmul(rbb[:], lhsT=ones64[:], rhs=rtmpb[:],
                                         start=True, stop=True)
                        rb2 = rcp.tile([64, 512], F32, tag="rb2", name="rb27")
                        nc.vector.reciprocal_approx_fast(out=rb2[:], in_=rbb[:])
                        nc.vector.tensor_mul(att_dst, av[0:64, :], rb2[:])
                        return last_pt
                    # copy psum accumulator out immediately so the bank frees
                    avc = rcp.tile([65, 512], F32, tag="avc", name=f"avc{h}")
                    rbc = rcp.tile([64, 512], F32, tag="rb", name=f"rb{h}")
                    rtmp = rcp.tile([1, 512], F32, tag="rt", name=f"rt{h}")
                    nc.vector.tensor_copy(avc[:], av[:])
                    nc.vector.tensor_copy(rtmp[:], av[64:65, :])
                    rb2 = rcp.tile([64, 512], F32, tag="rb2", name=f"rb2{h}")
                    nc.gpsimd.partition_broadcast(rbc[:], rtmp[:])
                    nc.vector.reciprocal_approx_fast(out=rb2[:], in_=rbc[:])
                    nc.vector.tensor_mul(att_dst, avc[0:64, :], rb2[:])

                # ---- scope A: q proj, K0, head 0 with V-proj dripped in ----
                pfx = tc.tile_pool(name="xin", bufs=1)
                xp = pfx.__enter__()
                scA = tc.tile_pool(name="ps_scA", bufs=2, space="PSUM")
                pssc2 = scA.__enter__()
                bgp_cm = tc.tile_pool(name="ps_bg", bufs=2, space="PSUM")
                bgp = bgp_cm.__enter__()

                Wq_sb = xp.tile([128, 2048], BF16, tag="wq")
                xqT_sb = xp.tile([128, 2048], BF16, tag="xq")
                vst = xp.tile([128, 4 * S], BF16, tag="vs", name="vstage")
                kst0 = ksp.tile([128, 4 * S], BF16, tag="ks", name="kstage0")
                # split first loads so q_proj can begin on the first halves;
                # K/V staging issues from otherwise-idle engine queues so the
                # sync engine's serial DMA-issue cost doesn't pace arrivals
                xq_d = xqT_sb[:].rearrange("p (k s) -> p k s", k=4)
                xq_s = xqT.ap().rearrange("(k p) s -> p k s", p=128)
                wq_d = Wq_sb[:].rearrange("p (k n) -> p k n", k=4)
                wq_s = Wq.ap().rearrange("(k p) n -> p k n", p=128)
                nc.sync.dma_start(xq_d[:, 0:2, :], xq_s[:, 0:2, :])
                nc.sync.dma_start(wq_d[:, 0:2, :], wq_s[:, 0:2, :])
                nc.sync.dma_start(xq_d[:, 2:4, :], xq_s[:, 2:4, :])
                nc.sync.dma_start(wq_d[:, 2:4, :], wq_s[:, 2:4, :])
                nc.sync.dma_start(
                    Wk_sb[:].rearrange("p (k n) -> p k n", k=4),
                    Wk.ap().rearrange("(k p) n -> p k n", p=128))
                nc.sync.dma_start(
                    Wv_sb[:].rearrange("p (k n) -> p k n", k=4),
                    Wv.ap().rearrange("(k p) n -> p k n", p=128))
                # K/V chunks interleaved to match head-0's consumption order:
                # chunk c of head 0 consumes kst c+1 and vst c//2
                kd = kst0[:].rearrange("p (k s) -> p k s", k=4)
                ks_ = keysT.ap().rearrange("(k p) s -> p k s", p=128)
                vd = vst[:].rearrange("p (k s) -> p k s", k=4)
                vs_ = valsT.ap().rearrange("(k p) s -> p k s", p=128)
                order = [(0, 0), (0, 1), (1, 0), (0, 2), (0, 3), (1, 1),
                         (0, 4), (0, 5), (1, 2), (0, 6), (0, 7), (1, 3),
                         (1, 4), (1, 5), (1, 6), (1, 7)]
                for which, ci in order:
                    dst, src = (kd, ks_) if which == 0 else (vd, vs_)
                    nc.sync.dma_start(dst[:, :, 512 * ci:512 * ci + 512],
                                      src[:, :, 512 * ci:512 * ci + 512])
                nc.sync.dma_start(
                    Wo_sb[:].rearrange("p (k n) -> p k n", k=4),
                    Wo.ap().rearrange("(k p) n -> p k n", p=128))
                bo_src = bo.ap().rearrange("(m p) -> p m", p=128)
                nc.sync.dma_start(boA[:], bo_src[:, 0:2])
                nc.sync.dma_start(boB[:], bo_src[:, 2:4])

                # PE p-state warmup: matmuls run at ~1.2 GHz until the PE has
                # been continuously busy ~3us, and the first real matmul can't
                # start until its DMA lands (~12us). These throwaway 256-col
                # matmuls are emitted BEFORE q_proj so they unconditionally
                # lead the PE stream, ramping the clock while DMA streams in.
                wrm_cm = tc.tile_pool(name="ps_warm", bufs=1, space="PSUM")
                wrm = wrm_cm.__enter__()
                warm_ps = wrm.tile([64, 256], F32, tag="w", name="warm_ps")
                for _ in range(20):
                    nc.tensor.matmul(
                        warm_ps[:], lhsT=warm_sb[:, 0:64],
                        rhs=warm_sb[:, 0:256], start=True, stop=True)
                wrm_cm.__exit__(None, None, None)

                q_proj(bgp)
                k_proj_group(0, 0, bgp, "bg")

                def drip_kv(c):
                    # K-pair-0 groups dripped just ahead of the chunks that
                    # need them (chunk c+1 needs sc <= (2c+3)//4 <= c+1), so
                    # the PE isn't parked on not-yet-loaded kst chunks
                    if c + 1 < 8:
                        k_proj_group(0, c + 1, bgp, "bg")
                    for j in (2 * c, 2 * c + 1):
                        if j < JT:
                            v_proj_group(j, bgp)

                attention_head(0, pssc2, 2, drip_kv)

                bgp_cm.__exit__(None, None, None)
                scA.__exit__(None, None, None)
                pfx.__exit__(None, None, None)

                # ---- scope B: heads 1-7; K pair m batched before head 2m;
                # fused fc (W2 = Wo@Wo folded on host) emitted in-scope so its
                # psum accumulators carve the scores-pool banks with no
                # scope-exit barrier: k=0..2 partials fill the PE gap while
                # head 7's normalization chain finishes attTp[3] ----
                with tc.tile_pool(name="ps_sc", bufs=2, space="PSUM") as pssc, \
                     tc.tile_pool(name="ps_kp", bufs=1, space="PSUM") as pskp:
                    # heads are ACT-paced (exp chain ~16.1us vs PE ~13.65us),
                    # so K pair m's 8 proj groups drip into the PE slack of
                    # heads 2m-1 (late chunks) and 2m (alternating chunks)
                    # instead of running as pure-PE batches between heads
                    sched = {h: {} for h in range(1, 8)}
                    for m in (1, 2, 3):
                        for i in range(4):
                            sched[2 * m - 1][7 + i] = (m, i)
                            sched[2 * m][2 + 2 * i] = (m, 4 + i)

                    def make_drip_k(h):
                        hs = sched.get(h, {})
                        if not hs:
                            return None
                        def drip(c):
                            if c in hs:
                                m, sc = hs[c]
                                k_proj_group(m, sc, pskp, "kp")
                        return drip

                    for h in range(1, 7):
                        attention_head(h, pssc, CHUNK, make_drip_k(h))
                    gate_pt = attention_head(7, pssc, CHUNK)

                    fcA = pssc.tile([128, 1536], F32, tag="sc", name="fcA")
                    fcB = pssc.tile([128, 1536], F32, tag="sc", name="fcB")
                    # gate the fc partials on head 7's last exp chunk: without
                    # this they become ready early and stuff the PE hardware
                    # queue ahead of head 7's final attn@V matmuls, delaying
                    # the whole normalization chain
                    nc.vector.tensor_copy(fcA[0:1, 0:1], gate_pt[0:1, 0:1])
                    nc.vector.tensor_copy(fcB[0:1, 0:1], gate_pt[0:1, 0:1])
                    fct = [fcA[:, 0:512], fcA[:, 512:1024],
                           fcB[:, 0:512], fcB[:, 512:1024]]
                    yT_d = yT.ap().rearrange("(m p) f -> p m f", m=4, p=128)
                    for m in range(4):
                        for k in range(3):
                            nc.tensor.matmul(
                                fct[m], lhsT=Wo_sb[:, 512 * k + 128 * m:512 * k + 128 * m + 128],
                                rhs=attTp[k][:], start=(k == 0), stop=False,
                                skip_group_check=True)
                    for m in range(4):
                        nc.tensor.matmul(
                            fct[m], lhsT=Wo_sb[:, 512 * 3 + 128 * m:512 * 3 + 128 * m + 128],
                            rhs=attTp[3][:], start=False, stop=True,
                            skip_group_check=True)
                    # biases paired same-engine per psum tile (fcA -> ACT,
                    # fcB -> DVE) with per-engine bias tiles: the tail stages
                    # share no tiles across engines, so nothing serializes
                    nc.scalar.add(o2p[0][:], fct[0], boA[:, 0:1])
                    nc.scalar.add(o2p[1][:], fct[1], boA[:, 1:2])
                    nc.vector.tensor_scalar_add(o2p[2][:], fct[2], boB[:, 0:1])
                    nc.vector.tensor_scalar_add(o2p[3][:], fct[3], boB[:, 1:2])
                    for m, eng in ((0, nc.sync), (1, nc.sync),
                                   (2, nc.gpsimd), (3, nc.gpsimd)):
                        eng.dma_start(yT_d[:, m, :], o2p[m][:])

    nc.compile()
    return nc


@functools.lru_cache(maxsize=1)
def _get_program():
    return _build_program()


def _make_in_maps(queries, keys, values, Wq, Wk, Wv, Wo, bo):
    q = np.asarray(queries, np.float32).reshape(S, D)
    kT = np.ascontiguousarray(np.asarray(keys, np.float32).reshape(S, D).T
                              ).astype(ml_dtypes.bfloat16)
    vT = np.ascontiguousarray(np.asarray(values, np.float32).reshape(S, D).T
                              ).astype(ml_dtypes.bfloat16)
    Wq = np.ascontiguousarray(np.asarray(Wq, np.float32)).astype(ml_dtypes.bfloat16)
    Wk = np.ascontiguousarray(np.asarray(Wk, np.float32)).astype(ml_dtypes.bfloat16)
    Wv = np.ascontiguousarray(np.asarray(Wv, np.float32)).astype(ml_dtypes.bfloat16)
    # fold the two fc_out applications into one: y = a@(Wo@Wo) + (bo@Wo + bo)
    Wo64 = np.asarray(Wo, np.float64)
    bo64 = np.asarray(bo, np.float64)
    Wo = np.ascontiguousarray(Wo64 @ Wo64).astype(ml_dtypes.bfloat16)
    bo = np.ascontiguousarray(bo64 @ Wo64 + bo64).astype(np.float32)
    in_maps = []
    for c in range(NCORES):
        in_maps.append({
            "xqT": np.ascontiguousarray(q[c * CH:(c + 1) * CH].T).astype(ml_dtypes.bfloat16),
            "keysT": kT, "valsT": vT,
            "Wq": Wq, "Wk": Wk, "Wv": Wv, "Wo": Wo, "bo": bo,
        })
    return in_maps


def _run(in_maps, **kw):
    nc = _get_program()
    return run_bass_kernel_spmd(nc, in_maps, core_ids=list(range(NCORES)), **kw)


def kernel(queries, keys, values, Wq, Wk, Wv, Wo, bo):
    res = _run(_make_in_maps(queries, keys, values, Wq, Wk, Wv, Wo, bo))
    out = np.concatenate([res.results[c]["yT"].T for c in range(NCORES)], axis=0)
    return out.reshape(1, S, D)


def run_traced(queries, keys, values, Wq, Wk, Wv, Wo, bo):
    """Like kernel() but with NTFF profiling; returns (output, BassKernelResults)."""
    import types
    import trn_agent_boot.trn_boot as _tb
    from concourse import bass_utils
    hook = _tb._ntff_profile_via_ctypes("/opt/axon/libaxon_pjrt.so")
    mod = types.ModuleType("antenv.axon_hooks")
    mod.get_axon_ntff_profile_hook = lambda: hook
    sys.modules["antenv.axon_hooks"] = mod
    bass_utils.upload_artifacts = lambda tmpdir: tmpdir
    res = _run(_make_in_maps(queries, keys, values, Wq, Wk, Wv, Wo, bo), trace=True)
    out = np.concatenate([res.results[c]["yT"].T for c in range(NCORES)], axis=0)
    return out.reshape(1, S, D), res
